# revision 11
# baseline (speedup 1.0000x reference)
"""Fused GroupNorm + self-attention + proj + residual block for TRN2.

Data-parallel over batch: core b computes batch element b (B=8 = 8 cores),
no collectives. Full inputs in, full output out.
"""

import os
import sys
from contextlib import ExitStack

for _p in ("/opt/trn_rl_repo", "/opt/pypackages"):
    if _p not in sys.path:
        sys.path.append(_p)

import numpy as np

import concourse.bass as bass
import concourse.tile as tile
from concourse import mybir

C = 128          # channels
N = 4096         # pixels (64*64)
GROUPS = 8
GSIZE = C // GROUPS
EPS = 1e-5
NCORES = 8
CHUNK = 512      # i-chunk width (query pixels per PSUM bank)
NCHUNK = N // CHUNK
JT = 128         # j-tile (key pixels per partition block)
NJT = N // JT    # 32
SGRP = 2         # j-tiles per S-psum group (exp granularity)
NGRP = NJT // SGRP

F32 = mybir.dt.float32
F32R = mybir.dt.float32r
BF16 = mybir.dt.bfloat16
AF = mybir.ActivationFunctionType
ALU = mybir.AluOpType


def _r(ap):
    """View an f32 AP as float32r for full-rate TensorE matmuls."""
    return ap.bitcast(F32R)


def attention_block_tile(tc, outs, ins):
    """Build the per-core kernel. outs/ins are dicts of DRAM APs."""
    nc = tc.nc
    x_d = ins["x"]          # [C, N] f32
    wqT_d = ins["wqT"]      # [C, C] (scaled by C^-0.5)
    wkT_d = ins["wkT"]      # [C, C]
    wvT_d = ins["wvT"]      # [C, C]
    projT_d = ins["projT"]  # [C, C]
    bq_d = ins["bq"]        # [C, 1] (scaled)
    pbe_d = ins["pbe"]      # [C, 1] proj bias + proj_w @ bv
    gnw_d = ins["gn_w"]     # [C, 1]
    gnb_d = ins["gn_b"]     # [C, 1]
    gind_d = ins["g_ind"]   # [C, GROUPS] = 1/GSIZE at [p, p//GSIZE]
    gbc_d = ins["g_bcast"]  # [GROUPS, C] = 1.0 at [g, p] where p//GSIZE==g
    out_d = outs["out"]     # [C, N] f32

    ctx = ExitStack()
    const = ctx.enter_context(tc.tile_pool(name="const", bufs=1))
    big = ctx.enter_context(tc.tile_pool(name="big", bufs=1))
    small = ctx.enter_context(tc.tile_pool(name="small", bufs=2))
    etile = ctx.enter_context(tc.tile_pool(name="etile", bufs=20))
    acc1 = ctx.enter_context(tc.tile_pool(name="acc1", bufs=10))
    acc2 = ctx.enter_context(tc.tile_pool(name="acc2", bufs=6))
    acc3 = ctx.enter_context(tc.tile_pool(name="acc3", bufs=4))
    acc4 = ctx.enter_context(tc.tile_pool(name="acc4", bufs=3))
    chunkp = ctx.enter_context(tc.tile_pool(name="chunkp", bufs=3))
    psum_s = ctx.enter_context(tc.tile_pool(name="psum_s", bufs=2, space="PSUM"))
    psum_o = ctx.enter_context(tc.tile_pool(name="psum_o", bufs=2, space="PSUM"))
    psum_m = ctx.enter_context(tc.tile_pool(name="psum_m", bufs=2, space="PSUM"))

    dma = nc.sync

    # ---- load constants ----
    wqT_f = const.tile([C, C], F32, tag="wqT_f")
    wkT_f = const.tile([C, C], F32, tag="wkT_f")
    wvT_f = const.tile([C, C], F32, tag="wvT_f")
    projT_f = const.tile([C, C], F32, tag="projT_f")
    dma.dma_start(out=wqT_f, in_=wqT_d)
    dma.dma_start(out=wkT_f, in_=wkT_d)
    dma.dma_start(out=wvT_f, in_=wvT_d)
    dma.dma_start(out=projT_f, in_=projT_d)
    wqT = const.tile([C, C], F32R, tag="wqT")
    wkT = const.tile([C, C], F32R, tag="wkT")
    wvT = const.tile([C, C], F32R, tag="wvT")
    projT = const.tile([C, C], F32R, tag="projT")
    nc.vector.tensor_copy(out=wqT, in_=wqT_f)
    nc.vector.tensor_copy(out=wkT, in_=wkT_f)
    nc.vector.tensor_copy(out=wvT, in_=wvT_f)
    nc.vector.tensor_copy(out=projT, in_=projT_f)
    bq = const.tile([C, 1], F32, tag="bq")
    pbe = const.tile([C, 1], F32, tag="pbe")
    gnw = const.tile([C, 1], F32, tag="gnw")
    gnb = const.tile([C, 1], F32, tag="gnb")
    dma.dma_start(out=bq, in_=bq_d)
    dma.dma_start(out=pbe, in_=pbe_d)
    dma.dma_start(out=gnw, in_=gnw_d)
    dma.dma_start(out=gnb, in_=gnb_d)
    gind = const.tile([C, GROUPS], F32, tag="gind")
    gbc = const.tile([GROUPS, C], F32, tag="gbc")
    dma.dma_start(out=gind, in_=gind_d)
    dma.dma_start(out=gbc, in_=gbc_d)
    ones_bf = const.tile([C, 1], BF16, tag="ones_bf")
    nc.vector.memset(ones_bf, 1.0)
    ones1 = const.tile([1, C], F32, tag="ones1")
    nc.vector.memset(ones1, 1.0)

    # ---- input ----
    x_sb = big.tile([C, N], F32, tag="x")
    dma.dma_start(out=x_sb, in_=x_d)

    # ---- GroupNorm stats ----
    stats = small.tile([C, 8, 6], F32, tag="gn_stats")
    for s in range(8):
        nc.vector.bn_stats(out=stats[:, s, :], in_=x_sb[:, s * 512:(s + 1) * 512])
    mv = small.tile([C, 2], F32, tag="gn_mv")
    nc.vector.bn_aggr(out=mv, in_=stats)
    # per-channel [mean, E[x^2]]
    stat2 = small.tile([C, 2], F32, tag="gn_stat2")
    nc.vector.tensor_copy(out=stat2[:, 0:1], in_=mv[:, 0:1])
    m2 = small.tile([C, 1], F32, tag="gn_m2")
    nc.vector.tensor_mul(out=m2, in0=mv[:, 0:1], in1=mv[:, 0:1])
    nc.vector.tensor_add(out=stat2[:, 1:2], in0=mv[:, 1:2], in1=m2)
    # reduce over group channels: [mean_g, E2_g] (gind carries the 1/16)
    gstats_ps = psum_m.tile([GROUPS, 2], F32, tag="m")
    nc.tensor.matmul(gstats_ps, lhsT=gind, rhs=stat2, start=True, stop=True)
    gstats = small.tile([GROUPS, 2], F32, tag="gn_gstats")
    nc.vector.tensor_copy(out=gstats, in_=gstats_ps)
    gm2 = small.tile([GROUPS, 1], F32, tag="gn_gm2")
    nc.vector.tensor_mul(out=gm2, in0=gstats[:, 0:1], in1=gstats[:, 0:1])
    gvar = small.tile([GROUPS, 1], F32, tag="gn_gvar")
    nc.vector.tensor_tensor(out=gvar, in0=gstats[:, 1:2], in1=gm2, op=ALU.subtract)
    # istd = exp(-0.5*ln(var+eps)); overwrite gstats[:,1]
    eps_t = const.tile([GROUPS, 1], F32, tag="eps")
    nc.vector.memset(eps_t, EPS)
    glnv = small.tile([GROUPS, 1], F32, tag="gn_glnv")
    nc.scalar.activation(out=glnv, in_=gvar, func=AF.Ln, bias=eps_t)
    nc.scalar.activation(out=gstats[:, 1:2], in_=glnv, func=AF.Exp, scale=-0.5)
    # broadcast per-group [mean, istd] back to channels
    chst_ps = psum_m.tile([C, 2], F32, tag="m")
    nc.tensor.matmul(chst_ps, lhsT=gbc, rhs=gstats, start=True, stop=True)
    chst = small.tile([C, 2], F32, tag="gn_chst")
    nc.vector.tensor_copy(out=chst, in_=chst_ps)
    scale = small.tile([C, 1], F32, tag="gn_scale")
    nc.vector.tensor_mul(out=scale, in0=gnw, in1=chst[:, 1:2])
    sm = small.tile([C, 1], F32, tag="gn_sm")
    nc.vector.tensor_mul(out=sm, in0=chst[:, 0:1], in1=scale)
    shift = small.tile([C, 1], F32, tag="gn_shift")
    nc.vector.tensor_tensor(out=shift, in0=gnb, in1=sm, op=ALU.subtract)

    # ---- h = x*scale + shift ----
    h_sb = big.tile([C, N], F32R, tag="h")
    nc.vector.tensor_scalar(out=h_sb, in0=x_sb, scalar1=scale, scalar2=shift,
                            op0=ALU.mult, op1=ALU.add)

    # ---- q, k [C, N]; vT [N(j), C] in 32 partition-tiles ----
    q_sb = big.tile([C, N], F32R, tag="q")
    k_sb = big.tile([C, N], F32R, tag="k")
    vT_sb = big.tile([C, NJT * C], BF16, tag="vT")  # [j_local, jt*C + c]
    for s in range(NCHUNK):
        sl = slice(s * CHUNK, (s + 1) * CHUNK)
        q_ps = psum_m.tile([C, CHUNK], F32, tag="m")
        nc.tensor.matmul(q_ps, lhsT=wqT, rhs=h_sb[:, sl], start=True, stop=True)
        nc.vector.tensor_scalar_add(out=q_sb[:, sl], in0=q_ps, scalar1=bq)
        k_ps = psum_m.tile([C, CHUNK], F32, tag="m")
        nc.tensor.matmul(k_ps, lhsT=wkT, rhs=h_sb[:, sl], start=True, stop=True)
        nc.scalar.copy(out=k_sb[:, sl], in_=k_ps)
    for nt in range(NJT):
        vt_ps = psum_m.tile([C, C], F32, tag="m")
        nc.tensor.matmul(vt_ps, lhsT=h_sb[:, nt * JT:(nt + 1) * JT], rhs=wvT,
                         start=True, stop=True)
        nc.vector.tensor_copy(out=vT_sb[:, nt * C:(nt + 1) * C], in_=vt_ps)

    # ---- attention, per i-chunk ----
    for ic in range(NCHUNK):
        isl = slice(ic * CHUNK, (ic + 1) * CHUNK)
        etiles = []
        for g in range(NGRP):
            s2_ps = psum_s.tile([C, SGRP * CHUNK], F32, tag="s2")
            for hh in range(SGRP):
                jt = g * SGRP + hh
                nc.tensor.matmul(s2_ps[:, hh * CHUNK:(hh + 1) * CHUNK],
                                 lhsT=k_sb[:, jt * JT:(jt + 1) * JT],
                                 rhs=q_sb[:, isl], start=True, stop=True)
            e = etile.tile([C, SGRP * CHUNK], BF16, tag="e")
            nc.scalar.activation(out=e, in_=s2_ps, func=AF.Exp)
            etiles.append(e)
        # PV accumulation over all 32 j-tiles
        o_ps = psum_o.tile([C, CHUNK], F32, tag="o")
        for jt in range(NJT):
            g, hh = jt // SGRP, jt % SGRP
            nc.tensor.matmul(o_ps,
                             lhsT=vT_sb[:, jt * C:(jt + 1) * C],
                             rhs=etiles[g][:, hh * CHUNK:(hh + 1) * CHUNK],
                             start=(jt == 0), stop=(jt == NJT - 1))
        # denominator: bf16 pairwise tree then PE partition-reduce
        l1 = []
        for a in range(8):
            t = acc1.tile([C, SGRP * CHUNK], BF16, tag="a1")
            nc.vector.tensor_add(out=t, in0=etiles[2 * a], in1=etiles[2 * a + 1])
            l1.append(t)
        l2 = []
        for a in range(4):
            t = acc2.tile([C, SGRP * CHUNK], BF16, tag="a2")
            nc.vector.tensor_add(out=t, in0=l1[2 * a], in1=l1[2 * a + 1])
            l2.append(t)
        l3 = []
        for a in range(2):
            t = acc3.tile([C, SGRP * CHUNK], BF16, tag="a3")
            nc.vector.tensor_add(out=t, in0=l2[2 * a], in1=l2[2 * a + 1])
            l3.append(t)
        l4 = acc4.tile([C, SGRP * CHUNK], BF16, tag="a4")
        nc.vector.tensor_add(out=l4, in0=l3[0], in1=l3[1])
        acc512 = acc4.tile([C, CHUNK], BF16, tag="a5")
        nc.vector.tensor_add(out=acc512, in0=l4[:, 0:CHUNK], in1=l4[:, CHUNK:2 * CHUNK])
        sums_ps = psum_m.tile([1, CHUNK], F32, tag="m")
        nc.tensor.matmul(sums_ps, lhsT=ones_bf, rhs=acc512, start=True, stop=True)
        # 1/sums via exp(-ln), broadcast to all partitions through TensorE
        lns = chunkp.tile([1, CHUNK], F32, tag="lns")
        nc.scalar.activation(out=lns, in_=sums_ps, func=AF.Ln)
        bc_ps = psum_m.tile([C, CHUNK], F32, tag="m")
        nc.tensor.matmul(bc_ps, lhsT=ones1, rhs=lns, start=True, stop=True)
        recipb = chunkp.tile([C, CHUNK], F32, tag="recipb")
        nc.scalar.activation(out=recipb, in_=bc_ps, func=AF.Exp, scale=-1.0)
        attn = chunkp.tile([C, CHUNK], F32R, tag="attn")
        nc.vector.tensor_tensor(out=attn, in0=o_ps, in1=recipb, op=ALU.mult)
        # proj + bias + residual
        p_ps = psum_m.tile([C, CHUNK], F32, tag="m")
        nc.tensor.matmul(p_ps, lhsT=projT, rhs=attn, start=True, stop=True)
        out_sb = chunkp.tile([C, CHUNK], F32, tag="out")
        nc.vector.scalar_tensor_tensor(out=out_sb, in0=p_ps, scalar=pbe,
                                       in1=x_sb[:, isl], op0=ALU.add, op1=ALU.add)
        dma.dma_start(out=out_d[:, isl], in_=out_sb)

    ctx.close()


def _host_consts(gn_w, gn_b, qkv_w, qkv_b, proj_w, proj_b):
    s = float(C) ** -0.5
    wq = qkv_w[0:C] * s
    bqv = (qkv_b[0:C] * s).reshape(C, 1)
    wk = qkv_w[C:2 * C]
    wv = qkv_w[2 * C:3 * C]
    bv = qkv_b[2 * C:3 * C]
    pbe = (proj_b + proj_w @ bv).reshape(C, 1)
    g_ind = np.zeros((C, GROUPS), np.float32)
    g_bc = np.zeros((GROUPS, C), np.float32)
    for p in range(C):
        g_ind[p, p // GSIZE] = 1.0 / GSIZE
        g_bc[p // GSIZE, p] = 1.0
    return {
        "wqT": np.ascontiguousarray(wq.T, np.float32),
        "wkT": np.ascontiguousarray(wk.T, np.float32),
        "wvT": np.ascontiguousarray(wv.T, np.float32),
        "projT": np.ascontiguousarray(proj_w.T, np.float32),
        "bq": bqv.astype(np.float32),
        "pbe": pbe.astype(np.float32),
        "gn_w": gn_w.reshape(C, 1).astype(np.float32),
        "gn_b": gn_b.reshape(C, 1).astype(np.float32),
        "g_ind": g_ind,
        "g_bcast": g_bc,
    }


_CACHE = {}


def _hoist_matmul_waits(nc):
    """The 64B ISA structs carry only one attached sync-wait — hoist extras
    into standalone EventSemaphore waits right before the instruction."""
    for fn in nc.m.functions:
        for blk in fn.blocks:
            il = blk.instructions
            out = []
            changed = False
            for ins in il:
                si = ins.sync_info
                if (not isinstance(ins, mybir.InstEventSemaphore)
                        and si is not None and len(si.on_wait) > 1):
                    for wi, w in enumerate(si.on_wait[1:]):
                        ev = mybir.InstEventSemaphore(
                            name=f"{ins.name}_hw{wi}", ins=[], outs=[],
                            sync_info=mybir.SyncInfo(on_wait=[w], on_update=[]))
                        ev.engine = ins.engine
                        out.append(ev)
                    ins.sync_info = mybir.SyncInfo(
                        on_wait=[si.on_wait[0]], on_update=si.on_update)
                    changed = True
                out.append(ins)
            if changed:
                il[:] = out


def _build_nc():
    if "nc" in _CACHE:
        return _CACHE["nc"]
    nc = bass.Bass("TRN2", target_bir_lowering=False, debug=False)
    ins = {}
    ins["x"] = nc.declare_dram_parameter("x", [C, N], F32, isOutput=False)[:]
    for nm, shp in [("wqT", [C, C]), ("wkT", [C, C]), ("wvT", [C, C]),
                    ("projT", [C, C]), ("bq", [C, 1]), ("pbe", [C, 1]),
                    ("gn_w", [C, 1]), ("gn_b", [C, 1]),
                    ("g_ind", [C, GROUPS]), ("g_bcast", [GROUPS, C])]:
        ins[nm] = nc.declare_dram_parameter(nm, shp, F32, isOutput=False)[:]
    out = nc.declare_dram_parameter("out", [C, N], F32, isOutput=True)[:]
    with tile.TileContext(nc) as tc:
        attention_block_tile(tc, {"out": out}, ins)
    _hoist_matmul_waits(nc)
    _CACHE["nc"] = nc
    return nc


LAST_EXEC_NS = None
LAST_RESULT = None


def _ensure_ntff_hook():
    """Provide antenv.axon_hooks (absent in this image) so trace=True works."""
    import types

    try:
        from antenv import axon_hooks  # noqa: F401
        return
    except ImportError:
        pass
    import antenv
    mod = types.ModuleType("antenv.axon_hooks")
    _hook = [None]
    mod.set_axon_ntff_profile_hook = lambda h: _hook.__setitem__(0, h)
    mod.get_axon_ntff_profile_hook = lambda: _hook[0]
    sys.modules["antenv.axon_hooks"] = mod
    antenv.axon_hooks = mod
    try:
        from trn_agent_boot.trn_boot import _ntff_profile_via_ctypes
        hook = _ntff_profile_via_ctypes("/opt/axon/libaxon_pjrt.so")
        mod.set_axon_ntff_profile_hook(hook)
    except Exception as e:  # hook stays None; concourse degrades gracefully
        print(f"ntff hook unavailable: {e}", file=sys.stderr)


def kernel(x, gn_w, gn_b, qkv_w, qkv_b, proj_w, proj_b):
    global LAST_EXEC_NS, LAST_RESULT
    from concourse.bass_utils import run_bass_kernel_spmd

    x = np.asarray(x, np.float32)
    B = x.shape[0]
    xf = x.reshape(B, C, N)
    consts = _host_consts(np.asarray(gn_w, np.float32), np.asarray(gn_b, np.float32),
                          np.asarray(qkv_w, np.float32), np.asarray(qkv_b, np.float32),
                          np.asarray(proj_w, np.float32), np.asarray(proj_b, np.float32))
    nc = _build_nc()
    in_maps = [dict(consts, x=np.ascontiguousarray(xf[b])) for b in range(NCORES)]
    trace = bool(int(os.environ.get("KERNEL_TRACE", "0")))
    if trace:
        _ensure_ntff_hook()
    res = run_bass_kernel_spmd(nc, in_maps, core_ids=list(range(NCORES)), trace=trace)
    LAST_EXEC_NS = getattr(res, "exec_time_ns", None)
    LAST_RESULT = res
    out = np.stack([res.results[b]["out"] for b in range(NCORES)], axis=0)
    return out.reshape(B, C, 64, 64).astype(np.float32)


# revision 12
# speedup vs baseline: 1.0160x; 1.0160x over previous
"""Fused GroupNorm + self-attention + proj + residual block for TRN2.

Data-parallel over batch: core b computes batch element b (B=8 = 8 cores),
no collectives. Full inputs in, full output out.
"""

import os
import sys
from contextlib import ExitStack

for _p in ("/opt/trn_rl_repo", "/opt/pypackages"):
    if _p not in sys.path:
        sys.path.append(_p)

import numpy as np

import concourse.bass as bass
import concourse.tile as tile
from concourse import mybir

C = 128          # channels
N = 4096         # pixels (64*64)
GROUPS = 8
GSIZE = C // GROUPS
EPS = 1e-5
NCORES = 8
CHUNK = 512      # i-chunk width (query pixels per PSUM bank)
NCHUNK = N // CHUNK
JT = 128         # j-tile (key pixels per partition block)
NJT = N // JT    # 32
SGRP = 2         # j-tiles per S-psum group (exp granularity)
NGRP = NJT // SGRP

F32 = mybir.dt.float32
F32R = mybir.dt.float32r
BF16 = mybir.dt.bfloat16
AF = mybir.ActivationFunctionType
ALU = mybir.AluOpType


def _r(ap):
    """View an f32 AP as float32r for full-rate TensorE matmuls."""
    return ap.bitcast(F32R)


def attention_block_tile(tc, outs, ins):
    """Build the per-core kernel. outs/ins are dicts of DRAM APs."""
    nc = tc.nc
    x_d = ins["x"]          # [C, N] f32
    wqT_d = ins["wqT"]      # [C, C] (scaled by C^-0.5)
    wkT_d = ins["wkT"]      # [C, C]
    wvT_d = ins["wvT"]      # [C, C]
    projT_d = ins["projT"]  # [C, C]
    bq_d = ins["bq"]        # [C, 1] (scaled)
    pbe_d = ins["pbe"]      # [C, 1] proj bias + proj_w @ bv
    gnw_d = ins["gn_w"]     # [C, 1]
    gnb_d = ins["gn_b"]     # [C, 1]
    gind_d = ins["g_ind"]   # [C, GROUPS] = 1/GSIZE at [p, p//GSIZE]
    gbc_d = ins["g_bcast"]  # [GROUPS, C] = 1.0 at [g, p] where p//GSIZE==g
    out_d = outs["out"]     # [C, N] f32

    ctx = ExitStack()
    const = ctx.enter_context(tc.tile_pool(name="const", bufs=1))
    big = ctx.enter_context(tc.tile_pool(name="big", bufs=1))
    small = ctx.enter_context(tc.tile_pool(name="small", bufs=2))
    etile = ctx.enter_context(tc.tile_pool(name="etile", bufs=24))
    acc1 = ctx.enter_context(tc.tile_pool(name="acc1", bufs=12))
    acc2 = ctx.enter_context(tc.tile_pool(name="acc2", bufs=6))
    acc3 = ctx.enter_context(tc.tile_pool(name="acc3", bufs=4))
    acc4 = ctx.enter_context(tc.tile_pool(name="acc4", bufs=3))
    chunkp = ctx.enter_context(tc.tile_pool(name="chunkp", bufs=4))
    psum_s = ctx.enter_context(tc.tile_pool(name="psum_s", bufs=2, space="PSUM"))
    psum_o = ctx.enter_context(tc.tile_pool(name="psum_o", bufs=2, space="PSUM"))
    psum_m = ctx.enter_context(tc.tile_pool(name="psum_m", bufs=2, space="PSUM"))

    dma = nc.sync

    # ---- load constants ----
    wqT_f = const.tile([C, C], F32, tag="wqT_f")
    wkT_f = const.tile([C, C], F32, tag="wkT_f")
    wvT_f = const.tile([C, C], F32, tag="wvT_f")
    projT_f = const.tile([C, C], F32, tag="projT_f")
    dma.dma_start(out=wqT_f, in_=wqT_d)
    dma.dma_start(out=wkT_f, in_=wkT_d)
    dma.dma_start(out=wvT_f, in_=wvT_d)
    dma.dma_start(out=projT_f, in_=projT_d)
    wqT = const.tile([C, C], F32R, tag="wqT")
    wkT = const.tile([C, C], F32R, tag="wkT")
    wvT = const.tile([C, C], F32R, tag="wvT")
    projT = const.tile([C, C], F32R, tag="projT")
    nc.vector.tensor_copy(out=wqT, in_=wqT_f)
    nc.vector.tensor_copy(out=wkT, in_=wkT_f)
    nc.vector.tensor_copy(out=wvT, in_=wvT_f)
    nc.vector.tensor_copy(out=projT, in_=projT_f)
    bq = const.tile([C, 1], F32, tag="bq")
    pbe = const.tile([C, 1], F32, tag="pbe")
    gnw = const.tile([C, 1], F32, tag="gnw")
    gnb = const.tile([C, 1], F32, tag="gnb")
    dma.dma_start(out=bq, in_=bq_d)
    dma.dma_start(out=pbe, in_=pbe_d)
    dma.dma_start(out=gnw, in_=gnw_d)
    dma.dma_start(out=gnb, in_=gnb_d)
    gind = const.tile([C, GROUPS], F32, tag="gind")
    gbc = const.tile([GROUPS, C], F32, tag="gbc")
    dma.dma_start(out=gind, in_=gind_d)
    dma.dma_start(out=gbc, in_=gbc_d)
    ones_bf = const.tile([C, 1], BF16, tag="ones_bf")
    nc.vector.memset(ones_bf, 1.0)
    ones1 = const.tile([1, C], F32, tag="ones1")
    nc.vector.memset(ones1, 1.0)

    # ---- input ----
    x_sb = big.tile([C, N], F32, tag="x")
    dma.dma_start(out=x_sb, in_=x_d)

    # ---- GroupNorm stats ----
    stats = small.tile([C, 8, 6], F32, tag="gn_stats")
    for s in range(8):
        nc.vector.bn_stats(out=stats[:, s, :], in_=x_sb[:, s * 512:(s + 1) * 512])
    mv = small.tile([C, 2], F32, tag="gn_mv")
    nc.vector.bn_aggr(out=mv, in_=stats)
    # per-channel [mean, E[x^2]]
    stat2 = small.tile([C, 2], F32, tag="gn_stat2")
    nc.vector.tensor_copy(out=stat2[:, 0:1], in_=mv[:, 0:1])
    m2 = small.tile([C, 1], F32, tag="gn_m2")
    nc.vector.tensor_mul(out=m2, in0=mv[:, 0:1], in1=mv[:, 0:1])
    nc.vector.tensor_add(out=stat2[:, 1:2], in0=mv[:, 1:2], in1=m2)
    # reduce over group channels: [mean_g, E2_g] (gind carries the 1/16)
    gstats_ps = psum_m.tile([GROUPS, 2], F32, tag="m")
    nc.tensor.matmul(gstats_ps, lhsT=gind, rhs=stat2, start=True, stop=True)
    gstats = small.tile([GROUPS, 2], F32, tag="gn_gstats")
    nc.vector.tensor_copy(out=gstats, in_=gstats_ps)
    gm2 = small.tile([GROUPS, 1], F32, tag="gn_gm2")
    nc.vector.tensor_mul(out=gm2, in0=gstats[:, 0:1], in1=gstats[:, 0:1])
    gvar = small.tile([GROUPS, 1], F32, tag="gn_gvar")
    nc.vector.tensor_tensor(out=gvar, in0=gstats[:, 1:2], in1=gm2, op=ALU.subtract)
    # istd = exp(-0.5*ln(var+eps)); overwrite gstats[:,1]
    eps_t = const.tile([GROUPS, 1], F32, tag="eps")
    nc.vector.memset(eps_t, EPS)
    glnv = small.tile([GROUPS, 1], F32, tag="gn_glnv")
    nc.scalar.activation(out=glnv, in_=gvar, func=AF.Ln, bias=eps_t)
    nc.scalar.activation(out=gstats[:, 1:2], in_=glnv, func=AF.Exp, scale=-0.5)
    # broadcast per-group [mean, istd] back to channels
    chst_ps = psum_m.tile([C, 2], F32, tag="m")
    nc.tensor.matmul(chst_ps, lhsT=gbc, rhs=gstats, start=True, stop=True)
    chst = small.tile([C, 2], F32, tag="gn_chst")
    nc.vector.tensor_copy(out=chst, in_=chst_ps)
    scale = small.tile([C, 1], F32, tag="gn_scale")
    nc.vector.tensor_mul(out=scale, in0=gnw, in1=chst[:, 1:2])
    sm = small.tile([C, 1], F32, tag="gn_sm")
    nc.vector.tensor_mul(out=sm, in0=chst[:, 0:1], in1=scale)
    shift = small.tile([C, 1], F32, tag="gn_shift")
    nc.vector.tensor_tensor(out=shift, in0=gnb, in1=sm, op=ALU.subtract)

    # ---- h = x*scale + shift ----
    h_sb = big.tile([C, N], F32R, tag="h")
    nc.vector.tensor_scalar(out=h_sb, in0=x_sb, scalar1=scale, scalar2=shift,
                            op0=ALU.mult, op1=ALU.add)

    # ---- q, k [C, N]; vT [N(j), C] in 32 partition-tiles ----
    q_sb = big.tile([C, N], BF16, tag="q")
    k_sb = big.tile([C, N], BF16, tag="k")
    vT_sb = big.tile([C, NJT * C], BF16, tag="vT")  # [j_local, jt*C + c]
    for s in range(NCHUNK):
        sl = slice(s * CHUNK, (s + 1) * CHUNK)
        q_ps = psum_m.tile([C, CHUNK], F32, tag="m")
        nc.tensor.matmul(q_ps, lhsT=wqT, rhs=h_sb[:, sl], start=True, stop=True)
        nc.vector.tensor_scalar_add(out=q_sb[:, sl], in0=q_ps, scalar1=bq)
        k_ps = psum_m.tile([C, CHUNK], F32, tag="m")
        nc.tensor.matmul(k_ps, lhsT=wkT, rhs=h_sb[:, sl], start=True, stop=True)
        nc.vector.tensor_copy(out=k_sb[:, sl], in_=k_ps)
    for nt in range(NJT):
        vt_ps = psum_m.tile([C, C], F32, tag="m")
        nc.tensor.matmul(vt_ps, lhsT=h_sb[:, nt * JT:(nt + 1) * JT], rhs=wvT,
                         start=True, stop=True)
        nc.vector.tensor_copy(out=vT_sb[:, nt * C:(nt + 1) * C], in_=vt_ps)

    # ---- attention, per i-chunk ----
    for ic in range(NCHUNK):
        isl = slice(ic * CHUNK, (ic + 1) * CHUNK)
        etiles = []
        for g in range(NGRP):
            s2_ps = psum_s.tile([C, SGRP * CHUNK], F32, tag="s2")
            for hh in range(SGRP):
                jt = g * SGRP + hh
                nc.tensor.matmul(s2_ps[:, hh * CHUNK:(hh + 1) * CHUNK],
                                 lhsT=k_sb[:, jt * JT:(jt + 1) * JT],
                                 rhs=q_sb[:, isl], start=True, stop=True)
            e = etile.tile([C, SGRP * CHUNK], BF16, tag="e")
            nc.scalar.activation(out=e, in_=s2_ps, func=AF.Exp)
            etiles.append(e)
        # PV accumulation over all 32 j-tiles
        o_ps = psum_o.tile([C, CHUNK], F32, tag="o")
        for jt in range(NJT):
            g, hh = jt // SGRP, jt % SGRP
            nc.tensor.matmul(o_ps,
                             lhsT=vT_sb[:, jt * C:(jt + 1) * C],
                             rhs=etiles[g][:, hh * CHUNK:(hh + 1) * CHUNK],
                             start=(jt == 0), stop=(jt == NJT - 1))
        # denominator: bf16 pairwise tree then PE partition-reduce
        l1 = []
        for a in range(8):
            t = acc1.tile([C, SGRP * CHUNK], BF16, tag="a1")
            nc.vector.tensor_add(out=t, in0=etiles[2 * a], in1=etiles[2 * a + 1])
            l1.append(t)
        l2 = []
        for a in range(4):
            t = acc2.tile([C, SGRP * CHUNK], BF16, tag="a2")
            nc.vector.tensor_add(out=t, in0=l1[2 * a], in1=l1[2 * a + 1])
            l2.append(t)
        l3 = []
        for a in range(2):
            t = acc3.tile([C, SGRP * CHUNK], BF16, tag="a3")
            nc.vector.tensor_add(out=t, in0=l2[2 * a], in1=l2[2 * a + 1])
            l3.append(t)
        l4 = acc4.tile([C, SGRP * CHUNK], BF16, tag="a4")
        nc.vector.tensor_add(out=l4, in0=l3[0], in1=l3[1])
        acc512 = acc4.tile([C, CHUNK], BF16, tag="a5")
        nc.vector.tensor_add(out=acc512, in0=l4[:, 0:CHUNK], in1=l4[:, CHUNK:2 * CHUNK])
        sums_ps = psum_m.tile([1, CHUNK], F32, tag="m")
        nc.tensor.matmul(sums_ps, lhsT=ones_bf, rhs=acc512, start=True, stop=True)
        # 1/sums via exp(-ln), broadcast to all partitions through TensorE
        lns = chunkp.tile([1, CHUNK], F32, tag="lns")
        nc.scalar.activation(out=lns, in_=sums_ps, func=AF.Ln)
        bc_ps = psum_m.tile([C, CHUNK], F32, tag="m")
        nc.tensor.matmul(bc_ps, lhsT=ones1, rhs=lns, start=True, stop=True)
        recipb = chunkp.tile([C, CHUNK], F32, tag="recipb")
        nc.scalar.activation(out=recipb, in_=bc_ps, func=AF.Exp, scale=-1.0)
        attn = chunkp.tile([C, CHUNK], F32R, tag="attn")
        nc.vector.tensor_copy(out=attn, in_=o_ps)
        # proj on unnormalized O; normalization commutes past the 1x1 conv
        p_ps = psum_m.tile([C, CHUNK], F32, tag="m")
        nc.tensor.matmul(p_ps, lhsT=projT, rhs=attn, start=True, stop=True)
        pn = chunkp.tile([C, CHUNK], F32, tag="pn")
        nc.vector.tensor_tensor(out=pn, in0=p_ps, in1=recipb, op=ALU.mult)
        out_sb = chunkp.tile([C, CHUNK], F32, tag="out")
        nc.vector.scalar_tensor_tensor(out=out_sb, in0=pn, scalar=pbe,
                                       in1=x_sb[:, isl], op0=ALU.add, op1=ALU.add)
        dma.dma_start(out=out_d[:, isl], in_=out_sb)

    ctx.close()


def _host_consts(gn_w, gn_b, qkv_w, qkv_b, proj_w, proj_b):
    s = float(C) ** -0.5
    wq = qkv_w[0:C] * s
    bqv = (qkv_b[0:C] * s).reshape(C, 1)
    wk = qkv_w[C:2 * C]
    wv = qkv_w[2 * C:3 * C]
    bv = qkv_b[2 * C:3 * C]
    pbe = (proj_b + proj_w @ bv).reshape(C, 1)
    g_ind = np.zeros((C, GROUPS), np.float32)
    g_bc = np.zeros((GROUPS, C), np.float32)
    for p in range(C):
        g_ind[p, p // GSIZE] = 1.0 / GSIZE
        g_bc[p // GSIZE, p] = 1.0
    return {
        "wqT": np.ascontiguousarray(wq.T, np.float32),
        "wkT": np.ascontiguousarray(wk.T, np.float32),
        "wvT": np.ascontiguousarray(wv.T, np.float32),
        "projT": np.ascontiguousarray(proj_w.T, np.float32),
        "bq": bqv.astype(np.float32),
        "pbe": pbe.astype(np.float32),
        "gn_w": gn_w.reshape(C, 1).astype(np.float32),
        "gn_b": gn_b.reshape(C, 1).astype(np.float32),
        "g_ind": g_ind,
        "g_bcast": g_bc,
    }


_CACHE = {}


def _hoist_matmul_waits(nc):
    """The 64B ISA structs carry only one attached sync-wait — hoist extras
    into standalone EventSemaphore waits right before the instruction."""
    for fn in nc.m.functions:
        for blk in fn.blocks:
            il = blk.instructions
            out = []
            changed = False
            for ins in il:
                si = ins.sync_info
                if (not isinstance(ins, mybir.InstEventSemaphore)
                        and si is not None and len(si.on_wait) > 1):
                    for wi, w in enumerate(si.on_wait[1:]):
                        ev = mybir.InstEventSemaphore(
                            name=f"{ins.name}_hw{wi}", ins=[], outs=[],
                            sync_info=mybir.SyncInfo(on_wait=[w], on_update=[]))
                        ev.engine = ins.engine
                        out.append(ev)
                    ins.sync_info = mybir.SyncInfo(
                        on_wait=[si.on_wait[0]], on_update=si.on_update)
                    changed = True
                out.append(ins)
            if changed:
                il[:] = out


def _build_nc():
    if "nc" in _CACHE:
        return _CACHE["nc"]
    nc = bass.Bass("TRN2", target_bir_lowering=False, debug=False)
    ins = {}
    ins["x"] = nc.declare_dram_parameter("x", [C, N], F32, isOutput=False)[:]
    for nm, shp in [("wqT", [C, C]), ("wkT", [C, C]), ("wvT", [C, C]),
                    ("projT", [C, C]), ("bq", [C, 1]), ("pbe", [C, 1]),
                    ("gn_w", [C, 1]), ("gn_b", [C, 1]),
                    ("g_ind", [C, GROUPS]), ("g_bcast", [GROUPS, C])]:
        ins[nm] = nc.declare_dram_parameter(nm, shp, F32, isOutput=False)[:]
    out = nc.declare_dram_parameter("out", [C, N], F32, isOutput=True)[:]
    with tile.TileContext(nc) as tc:
        attention_block_tile(tc, {"out": out}, ins)
    _hoist_matmul_waits(nc)
    _CACHE["nc"] = nc
    return nc


LAST_EXEC_NS = None
LAST_RESULT = None


def _ensure_ntff_hook():
    """Provide antenv.axon_hooks (absent in this image) so trace=True works."""
    import types

    try:
        from antenv import axon_hooks  # noqa: F401
        return
    except ImportError:
        pass
    import antenv
    mod = types.ModuleType("antenv.axon_hooks")
    _hook = [None]
    mod.set_axon_ntff_profile_hook = lambda h: _hook.__setitem__(0, h)
    mod.get_axon_ntff_profile_hook = lambda: _hook[0]
    sys.modules["antenv.axon_hooks"] = mod
    antenv.axon_hooks = mod
    try:
        from trn_agent_boot.trn_boot import _ntff_profile_via_ctypes
        hook = _ntff_profile_via_ctypes("/opt/axon/libaxon_pjrt.so")
        mod.set_axon_ntff_profile_hook(hook)
    except Exception as e:  # hook stays None; concourse degrades gracefully
        print(f"ntff hook unavailable: {e}", file=sys.stderr)


def kernel(x, gn_w, gn_b, qkv_w, qkv_b, proj_w, proj_b):
    global LAST_EXEC_NS, LAST_RESULT
    from concourse.bass_utils import run_bass_kernel_spmd

    x = np.asarray(x, np.float32)
    B = x.shape[0]
    xf = x.reshape(B, C, N)
    consts = _host_consts(np.asarray(gn_w, np.float32), np.asarray(gn_b, np.float32),
                          np.asarray(qkv_w, np.float32), np.asarray(qkv_b, np.float32),
                          np.asarray(proj_w, np.float32), np.asarray(proj_b, np.float32))
    nc = _build_nc()
    in_maps = [dict(consts, x=np.ascontiguousarray(xf[b])) for b in range(NCORES)]
    trace = bool(int(os.environ.get("KERNEL_TRACE", "0")))
    if trace:
        _ensure_ntff_hook()
    res = run_bass_kernel_spmd(nc, in_maps, core_ids=list(range(NCORES)), trace=trace)
    LAST_EXEC_NS = getattr(res, "exec_time_ns", None)
    LAST_RESULT = res
    out = np.stack([res.results[b]["out"] for b in range(NCORES)], axis=0)
    return out.reshape(B, C, 64, 64).astype(np.float32)


# revision 13
# speedup vs baseline: 1.0172x; 1.0012x over previous
"""Fused GroupNorm + self-attention + proj + residual block for TRN2.

Data-parallel over batch: core b computes batch element b (B=8 = 8 cores),
no collectives. Full inputs in, full output out.
"""

import os
import sys
from contextlib import ExitStack

for _p in ("/opt/trn_rl_repo", "/opt/pypackages"):
    if _p not in sys.path:
        sys.path.append(_p)

import numpy as np

import concourse.bass as bass
import concourse.tile as tile
from concourse import mybir

C = 128          # channels
N = 4096         # pixels (64*64)
GROUPS = 8
GSIZE = C // GROUPS
EPS = 1e-5
NCORES = 8
CHUNK = 512      # i-chunk width (query pixels per PSUM bank)
NCHUNK = N // CHUNK
JT = 128         # j-tile (key pixels per partition block)
NJT = N // JT    # 32
SGRP = 2         # j-tiles per S-psum group (exp granularity)
NGRP = NJT // SGRP

F32 = mybir.dt.float32
F32R = mybir.dt.float32r
BF16 = mybir.dt.bfloat16
AF = mybir.ActivationFunctionType
ALU = mybir.AluOpType


def _r(ap):
    """View an f32 AP as float32r for full-rate TensorE matmuls."""
    return ap.bitcast(F32R)


def attention_block_tile(tc, outs, ins):
    """Build the per-core kernel. outs/ins are dicts of DRAM APs."""
    nc = tc.nc
    x_d = ins["x"]          # [C, N] f32
    wqT_d = ins["wqT"]      # [C, C] (scaled by C^-0.5)
    wkT_d = ins["wkT"]      # [C, C]
    wvT_d = ins["wvT"]      # [C, C]
    projT_d = ins["projT"]  # [C, C]
    bq_d = ins["bq"]        # [C, 1] (scaled)
    pbe_d = ins["pbe"]      # [C, 1] proj bias + proj_w @ bv
    gnw_d = ins["gn_w"]     # [C, 1]
    gnb_d = ins["gn_b"]     # [C, 1]
    gind_d = ins["g_ind"]   # [C, GROUPS] = 1/GSIZE at [p, p//GSIZE]
    gbc_d = ins["g_bcast"]  # [GROUPS, C] = 1.0 at [g, p] where p//GSIZE==g
    out_d = outs["out"]     # [C, N] f32

    ctx = ExitStack()
    const = ctx.enter_context(tc.tile_pool(name="const", bufs=1))
    big = ctx.enter_context(tc.tile_pool(name="big", bufs=1))
    small = ctx.enter_context(tc.tile_pool(name="small", bufs=2))
    etile = ctx.enter_context(tc.tile_pool(name="etile", bufs=24))
    acc1 = ctx.enter_context(tc.tile_pool(name="acc1", bufs=12))
    acc2 = ctx.enter_context(tc.tile_pool(name="acc2", bufs=6))
    acc3 = ctx.enter_context(tc.tile_pool(name="acc3", bufs=4))
    acc4 = ctx.enter_context(tc.tile_pool(name="acc4", bufs=3))
    chunkp = ctx.enter_context(tc.tile_pool(name="chunkp", bufs=4))
    psum_s = ctx.enter_context(tc.tile_pool(name="psum_s", bufs=2, space="PSUM"))
    psum_o = ctx.enter_context(tc.tile_pool(name="psum_o", bufs=2, space="PSUM"))
    psum_m = ctx.enter_context(tc.tile_pool(name="psum_m", bufs=2, space="PSUM"))

    dma = nc.sync

    # ---- load constants ----
    wqT_f = const.tile([C, C], F32, tag="wqT_f")
    wkT_f = const.tile([C, C], F32, tag="wkT_f")
    wvT_f = const.tile([C, C], F32, tag="wvT_f")
    projT_f = const.tile([C, C], F32, tag="projT_f")
    dma.dma_start(out=wqT_f, in_=wqT_d)
    dma.dma_start(out=wkT_f, in_=wkT_d)
    dma.dma_start(out=wvT_f, in_=wvT_d)
    dma.dma_start(out=projT_f, in_=projT_d)
    wqT = const.tile([C, C], F32R, tag="wqT")
    wkT = const.tile([C, C], F32R, tag="wkT")
    wvT = const.tile([C, C], F32R, tag="wvT")
    projT = const.tile([C, C], BF16, tag="projT")
    nc.vector.tensor_copy(out=wqT, in_=wqT_f)
    nc.vector.tensor_copy(out=wkT, in_=wkT_f)
    nc.vector.tensor_copy(out=wvT, in_=wvT_f)
    nc.vector.tensor_copy(out=projT, in_=projT_f)
    bq = const.tile([C, 1], F32, tag="bq")
    pbe = const.tile([C, 1], F32, tag="pbe")
    gnw = const.tile([C, 1], F32, tag="gnw")
    gnb = const.tile([C, 1], F32, tag="gnb")
    dma.dma_start(out=bq, in_=bq_d)
    dma.dma_start(out=pbe, in_=pbe_d)
    dma.dma_start(out=gnw, in_=gnw_d)
    dma.dma_start(out=gnb, in_=gnb_d)
    gind = const.tile([C, GROUPS], F32, tag="gind")
    gbc = const.tile([GROUPS, C], F32, tag="gbc")
    dma.dma_start(out=gind, in_=gind_d)
    dma.dma_start(out=gbc, in_=gbc_d)
    ones_bf = const.tile([C, 1], BF16, tag="ones_bf")
    nc.vector.memset(ones_bf, 1.0)
    ones1 = const.tile([1, C], F32, tag="ones1")
    nc.vector.memset(ones1, 1.0)
    ones_r = const.tile([C, 1], F32R, tag="ones_r")
    nc.vector.tensor_copy(out=ones_r, in_=ones_bf)
    warm = const.tile([1, 1], F32, tag="warm")
    nc.vector.memset(warm, 1.0)
    nc.scalar.activation(out=warm, in_=warm, func=AF.Ln)

    # ---- input (chunked DMA, stats pipelined) ----
    x_sb = big.tile([C, N], F32, tag="x")
    stats = small.tile([C, 8, 6], F32, tag="gn_stats")
    for s in range(8):
        nc.sync.dma_start(out=x_sb[:, s * 512:(s + 1) * 512],
                          in_=x_d[:, s * 512:(s + 1) * 512])
        nc.vector.bn_stats(out=stats[:, s, :], in_=x_sb[:, s * 512:(s + 1) * 512])
    mv = small.tile([C, 2], F32, tag="gn_mv")
    nc.vector.bn_aggr(out=mv, in_=stats)
    # per-channel [mean, E[x^2]]
    stat2 = small.tile([C, 2], F32, tag="gn_stat2")
    nc.vector.tensor_copy(out=stat2[:, 0:1], in_=mv[:, 0:1])
    m2 = small.tile([C, 1], F32, tag="gn_m2")
    nc.vector.tensor_mul(out=m2, in0=mv[:, 0:1], in1=mv[:, 0:1])
    nc.vector.tensor_add(out=stat2[:, 1:2], in0=mv[:, 1:2], in1=m2)
    # reduce over group channels: [mean_g, E2_g] (gind carries the 1/16)
    gstats_ps = psum_m.tile([GROUPS, 2], F32, tag="m")
    nc.tensor.matmul(gstats_ps, lhsT=gind, rhs=stat2, start=True, stop=True)
    gstats = small.tile([GROUPS, 2], F32, tag="gn_gstats")
    nc.vector.tensor_copy(out=gstats, in_=gstats_ps)
    gm2 = small.tile([GROUPS, 1], F32, tag="gn_gm2")
    nc.vector.tensor_mul(out=gm2, in0=gstats[:, 0:1], in1=gstats[:, 0:1])
    gvar = small.tile([GROUPS, 1], F32, tag="gn_gvar")
    nc.vector.tensor_tensor(out=gvar, in0=gstats[:, 1:2], in1=gm2, op=ALU.subtract)
    # istd = exp(-0.5*ln(var+eps)); overwrite gstats[:,1]
    eps_t = const.tile([GROUPS, 1], F32, tag="eps")
    nc.vector.memset(eps_t, EPS)
    glnv = small.tile([GROUPS, 1], F32, tag="gn_glnv")
    nc.scalar.activation(out=glnv, in_=gvar, func=AF.Ln, bias=eps_t)
    nc.scalar.activation(out=gstats[:, 1:2], in_=glnv, func=AF.Exp, scale=-0.5)
    # broadcast per-group [mean, istd] back to channels
    chst_ps = psum_m.tile([C, 2], F32, tag="m")
    nc.tensor.matmul(chst_ps, lhsT=gbc, rhs=gstats, start=True, stop=True)
    chst = small.tile([C, 2], F32, tag="gn_chst")
    nc.vector.tensor_copy(out=chst, in_=chst_ps)
    scale = small.tile([C, 1], F32, tag="gn_scale")
    nc.vector.tensor_mul(out=scale, in0=gnw, in1=chst[:, 1:2])
    sm = small.tile([C, 1], F32, tag="gn_sm")
    nc.vector.tensor_mul(out=sm, in0=chst[:, 0:1], in1=scale)
    shift = small.tile([C, 1], F32, tag="gn_shift")
    nc.vector.tensor_tensor(out=shift, in0=gnb, in1=sm, op=ALU.subtract)

    # ---- h, q, k, vT pipelined per chunk ----
    h_sb = big.tile([C, N], F32R, tag="h")
    q_sb = big.tile([C, N], BF16, tag="q")
    k_sb = big.tile([C, N], BF16, tag="k")
    vT_sb = big.tile([C, NJT * C], BF16, tag="vT")  # [j_local, jt*C + c]
    for s in range(NCHUNK):
        sl = slice(s * CHUNK, (s + 1) * CHUNK)
        nc.vector.tensor_scalar(out=h_sb[:, sl], in0=x_sb[:, sl], scalar1=scale,
                                scalar2=shift, op0=ALU.mult, op1=ALU.add)
        q_ps = psum_s.tile([C, CHUNK], F32, tag="s2")
        nc.tensor.matmul(q_ps, lhsT=wqT, rhs=h_sb[:, sl], start=True, stop=True)
        nc.vector.tensor_scalar_add(out=q_sb[:, sl], in0=q_ps, scalar1=bq)
        k_ps = psum_s.tile([C, CHUNK], F32, tag="s2")
        nc.tensor.matmul(k_ps, lhsT=wkT, rhs=h_sb[:, sl], start=True, stop=True)
        nc.scalar.copy(out=k_sb[:, sl], in_=k_ps)
        for nt in range(4 * s, 4 * s + 4):
            vt_ps = psum_m.tile([C, C], F32, tag="m")
            nc.tensor.matmul(vt_ps, lhsT=h_sb[:, nt * JT:(nt + 1) * JT], rhs=wvT,
                             start=True, stop=True)
            nc.scalar.copy(out=vT_sb[:, nt * C:(nt + 1) * C], in_=vt_ps)

    # ---- attention, per i-chunk ----
    for ic in range(NCHUNK):
        isl = slice(ic * CHUNK, (ic + 1) * CHUNK)
        etiles = []
        for g in range(NGRP):
            s2_ps = psum_s.tile([C, SGRP * CHUNK], F32, tag="s2")
            for hh in range(SGRP):
                jt = g * SGRP + hh
                nc.tensor.matmul(s2_ps[:, hh * CHUNK:(hh + 1) * CHUNK],
                                 lhsT=k_sb[:, jt * JT:(jt + 1) * JT],
                                 rhs=q_sb[:, isl], start=True, stop=True)
            e = etile.tile([C, SGRP * CHUNK], BF16, tag="e")
            nc.scalar.activation(out=e, in_=s2_ps, func=AF.Exp)
            etiles.append(e)
        # PV accumulation over all 32 j-tiles
        o_ps = psum_o.tile([C, CHUNK], F32, tag="o")
        for jt in range(NJT):
            g, hh = jt // SGRP, jt % SGRP
            nc.tensor.matmul(o_ps,
                             lhsT=vT_sb[:, jt * C:(jt + 1) * C],
                             rhs=etiles[g][:, hh * CHUNK:(hh + 1) * CHUNK],
                             start=(jt == 0), stop=(jt == NJT - 1))
        # denominator: bf16 pairwise tree then PE partition-reduce
        l1 = []
        for a in range(8):
            t = acc1.tile([C, SGRP * CHUNK], BF16, tag="a1")
            nc.vector.tensor_add(out=t, in0=etiles[2 * a], in1=etiles[2 * a + 1])
            l1.append(t)
        l2 = []
        for a in range(4):
            t = acc2.tile([C, SGRP * CHUNK], BF16, tag="a2")
            nc.vector.tensor_add(out=t, in0=l1[2 * a], in1=l1[2 * a + 1])
            l2.append(t)
        l3 = []
        for a in range(2):
            t = acc3.tile([C, SGRP * CHUNK], BF16, tag="a3")
            nc.vector.tensor_add(out=t, in0=l2[2 * a], in1=l2[2 * a + 1])
            l3.append(t)
        l4 = acc4.tile([C, SGRP * CHUNK], BF16, tag="a4")
        nc.vector.tensor_add(out=l4, in0=l3[0], in1=l3[1])
        acc512 = acc4.tile([C, CHUNK], F32R, tag="a5")
        nc.vector.tensor_add(out=acc512, in0=l4[:, 0:CHUNK], in1=l4[:, CHUNK:2 * CHUNK])
        sums_ps = psum_m.tile([1, CHUNK], F32, tag="m")
        nc.tensor.matmul(sums_ps, lhsT=ones_r, rhs=acc512, start=True, stop=True)
        # 1/sums via exp(-ln), broadcast to all partitions through TensorE
        lns = chunkp.tile([1, CHUNK], F32, tag="lns")
        nc.scalar.activation(out=lns, in_=sums_ps, func=AF.Ln)
        bc_ps = psum_m.tile([C, CHUNK], F32, tag="m")
        nc.tensor.matmul(bc_ps, lhsT=ones1, rhs=lns, start=True, stop=True)
        recipb = chunkp.tile([C, CHUNK], F32, tag="recipb")
        nc.scalar.activation(out=recipb, in_=bc_ps, func=AF.Exp, scale=-1.0)
        attn = chunkp.tile([C, CHUNK], BF16, tag="attn")
        nc.vector.tensor_copy(out=attn, in_=o_ps)
        # proj on unnormalized O; normalization commutes past the 1x1 conv
        p_ps = psum_m.tile([C, CHUNK], F32, tag="m")
        nc.tensor.matmul(p_ps, lhsT=projT, rhs=attn, start=True, stop=True)
        pn = chunkp.tile([C, CHUNK], F32, tag="pn")
        nc.vector.tensor_tensor(out=pn, in0=p_ps, in1=recipb, op=ALU.mult)
        out_sb = chunkp.tile([C, CHUNK], F32, tag="out")
        nc.vector.scalar_tensor_tensor(out=out_sb, in0=pn, scalar=pbe,
                                       in1=x_sb[:, isl], op0=ALU.add, op1=ALU.add)
        dma.dma_start(out=out_d[:, isl], in_=out_sb)

    ctx.close()


def _host_consts(gn_w, gn_b, qkv_w, qkv_b, proj_w, proj_b):
    s = float(C) ** -0.5
    wq = qkv_w[0:C] * s
    bqv = (qkv_b[0:C] * s).reshape(C, 1)
    wk = qkv_w[C:2 * C]
    wv = qkv_w[2 * C:3 * C]
    bv = qkv_b[2 * C:3 * C]
    pbe = (proj_b + proj_w @ bv).reshape(C, 1)
    g_ind = np.zeros((C, GROUPS), np.float32)
    g_bc = np.zeros((GROUPS, C), np.float32)
    for p in range(C):
        g_ind[p, p // GSIZE] = 1.0 / GSIZE
        g_bc[p // GSIZE, p] = 1.0
    return {
        "wqT": np.ascontiguousarray(wq.T, np.float32),
        "wkT": np.ascontiguousarray(wk.T, np.float32),
        "wvT": np.ascontiguousarray(wv.T, np.float32),
        "projT": np.ascontiguousarray(proj_w.T, np.float32),
        "bq": bqv.astype(np.float32),
        "pbe": pbe.astype(np.float32),
        "gn_w": gn_w.reshape(C, 1).astype(np.float32),
        "gn_b": gn_b.reshape(C, 1).astype(np.float32),
        "g_ind": g_ind,
        "g_bcast": g_bc,
    }


_CACHE = {}


def _hoist_matmul_waits(nc):
    """The 64B ISA structs carry only one attached sync-wait — hoist extras
    into standalone EventSemaphore waits right before the instruction."""
    for fn in nc.m.functions:
        for blk in fn.blocks:
            il = blk.instructions
            out = []
            changed = False
            for ins in il:
                si = ins.sync_info
                if (not isinstance(ins, mybir.InstEventSemaphore)
                        and si is not None and len(si.on_wait) > 1):
                    for wi, w in enumerate(si.on_wait[1:]):
                        ev = mybir.InstEventSemaphore(
                            name=f"{ins.name}_hw{wi}", ins=[], outs=[],
                            sync_info=mybir.SyncInfo(on_wait=[w], on_update=[]))
                        ev.engine = ins.engine
                        out.append(ev)
                    ins.sync_info = mybir.SyncInfo(
                        on_wait=[si.on_wait[0]], on_update=si.on_update)
                    changed = True
                out.append(ins)
            if changed:
                il[:] = out


def _build_nc():
    if "nc" in _CACHE:
        return _CACHE["nc"]
    nc = bass.Bass("TRN2", target_bir_lowering=False, debug=False)
    ins = {}
    ins["x"] = nc.declare_dram_parameter("x", [C, N], F32, isOutput=False)[:]
    for nm, shp in [("wqT", [C, C]), ("wkT", [C, C]), ("wvT", [C, C]),
                    ("projT", [C, C]), ("bq", [C, 1]), ("pbe", [C, 1]),
                    ("gn_w", [C, 1]), ("gn_b", [C, 1]),
                    ("g_ind", [C, GROUPS]), ("g_bcast", [GROUPS, C])]:
        ins[nm] = nc.declare_dram_parameter(nm, shp, F32, isOutput=False)[:]
    out = nc.declare_dram_parameter("out", [C, N], F32, isOutput=True)[:]
    with tile.TileContext(nc) as tc:
        attention_block_tile(tc, {"out": out}, ins)
    _hoist_matmul_waits(nc)
    _CACHE["nc"] = nc
    return nc


LAST_EXEC_NS = None
LAST_RESULT = None


def _ensure_ntff_hook():
    """Provide antenv.axon_hooks (absent in this image) so trace=True works."""
    import types

    try:
        from antenv import axon_hooks  # noqa: F401
        return
    except ImportError:
        pass
    import antenv
    mod = types.ModuleType("antenv.axon_hooks")
    _hook = [None]
    mod.set_axon_ntff_profile_hook = lambda h: _hook.__setitem__(0, h)
    mod.get_axon_ntff_profile_hook = lambda: _hook[0]
    sys.modules["antenv.axon_hooks"] = mod
    antenv.axon_hooks = mod
    try:
        from trn_agent_boot.trn_boot import _ntff_profile_via_ctypes
        hook = _ntff_profile_via_ctypes("/opt/axon/libaxon_pjrt.so")
        mod.set_axon_ntff_profile_hook(hook)
    except Exception as e:  # hook stays None; concourse degrades gracefully
        print(f"ntff hook unavailable: {e}", file=sys.stderr)


def kernel(x, gn_w, gn_b, qkv_w, qkv_b, proj_w, proj_b):
    global LAST_EXEC_NS, LAST_RESULT
    from concourse.bass_utils import run_bass_kernel_spmd

    x = np.asarray(x, np.float32)
    B = x.shape[0]
    xf = x.reshape(B, C, N)
    consts = _host_consts(np.asarray(gn_w, np.float32), np.asarray(gn_b, np.float32),
                          np.asarray(qkv_w, np.float32), np.asarray(qkv_b, np.float32),
                          np.asarray(proj_w, np.float32), np.asarray(proj_b, np.float32))
    nc = _build_nc()
    in_maps = [dict(consts, x=np.ascontiguousarray(xf[b])) for b in range(NCORES)]
    trace = bool(int(os.environ.get("KERNEL_TRACE", "0")))
    if trace:
        _ensure_ntff_hook()
    res = run_bass_kernel_spmd(nc, in_maps, core_ids=list(range(NCORES)), trace=trace)
    LAST_EXEC_NS = getattr(res, "exec_time_ns", None)
    LAST_RESULT = res
    out = np.stack([res.results[b]["out"] for b in range(NCORES)], axis=0)
    return out.reshape(B, C, 64, 64).astype(np.float32)
